# revision 2
# baseline (speedup 1.0000x reference)
"""GAT layer kernel for 8 Trainium2 NeuronCores.

Strategy (dst-sharded, fully core-independent — no collectives):

Host: cast x to fp16; each core owns a 12544-node dst slab. Per core, its
in-edges are bucketed into (dst-node, src-chunk) "slots" (4 chunks of
25088 table rows keep dma_gather's int16 indices in range; a chunk is
exactly 2 slabs, so every core's dst nodes live in one chunk). Slots are
sorted by length and packed 128-at-a-time into groups of uniform width
k_g; groups of one chunk are batched into big dma_gather calls. Group
widths/batching are shared across cores (single SPMD program); per-core
index DATA differs.

Device phase A (replicated): project all nodes h0 = gelu(x@w_in + b_in),
write one fp16 256B table row per node: [z(64) | s=z@a[:64] | d=z@a[64:]
| pad]. DMA-transpose loads feed PE matmuls directly.

Device phase B: per batch, one dma_gather pulls table rows for all edge
slots (edges land [slot-partition, column, 128]). Scalar engine computes
w = exp(lrelu(s_src + d_dst)) with the per-slot d as activation bias and
accumulates the softmax denominator for free; vector engine scales z by
w and segment-reduces along the free dim. Per-slot d rows come from one
dma_gather per chunk, where inactive chunks (per-core data: all-(-1)
indices + num_idxs_reg=0) generate zero descriptors.

Output rows are [U | denom] per slot; host scatter-adds slots onto nodes
(chunk partial sums recombine exactly) and divides.

Skipping the segment max: exp(lrelu(e)) is shift-invariant softmax math
and |e| <~ 2 here, so it is numerically safe and matches the reference.
"""

import sys

sys.path.insert(0, "/opt/trn_rl_repo")

import numpy as np

import concourse.bass as bass
import concourse.mybir as mybir
import concourse.tile as tile
from concourse import bacc
from concourse.bass_utils import run_bass_kernel_spmd
from concourse.vector_clock import ScopedClock

P = 128
SENT_S = -60000.0  # sentinel s: exp(lrelu(s + d)) == 0 in fp32
F16 = mybir.dt.float16
F32 = mybir.dt.float32
I16 = mybir.dt.int16
I32 = mybir.dt.int32
AF = mybir.ActivationFunctionType
ALU = mybir.AluOpType


def _patch_tile_drain():
    """Walrus in this container accepts at most ONE sync-wait command per
    instruction; Tile's tail drain waits on every allocated semaphore.
    Spread the drain waits over a chain of sync-engine NOPs (program order
    on one engine preserves the barrier)."""
    if getattr(tile.TileContext, "_drain_patched", False):
        return

    def _drain_and_barrier(self, tick_clock, wait_clock):
        collector = self.nc.sync.nop()
        wait_clock.add_sem_waits(
            collector.ins, ScopedClock({None: tick_clock.global_clock})
        )
        si = collector.ins.sync_info
        waits = list(si.on_wait) if si is not None else []
        if si is not None:
            si.on_wait = waits[:1]
        for i in range(1, len(waits)):
            nop = self.nc.sync.nop()
            nop.ins.sync_info = mybir.SyncInfo(on_wait=[waits[i]], on_update=[])
        self.nc.sync.drain()
        self.nc.all_engine_barrier()
        assert self.sems is not None
        popped = self.nc._tile_sem_poison_stack.pop()
        assert popped is self._sem_poison
        self.nc.clear_and_free_semaphores(list(self.sems.allocated().values()))
        self.nc.all_engine_barrier()

    tile.TileContext._drain_and_barrier = _drain_and_barrier
    tile.TileContext._drain_patched = True


def _split_sync_waits(nc: bass.Bass):
    """Post-pass (run after finalize/compile): any instruction carrying >1
    sync waits gets its extra waits hoisted into same-engine NOPs inserted
    immediately before it (same basic block, so per-engine program order
    is preserved)."""
    n = 0
    for f in nc.m.functions:
        for bb in f.blocks:
            insts = list(bb.instructions)
            out = []
            changed = False
            for ins in insts:
                si = ins.sync_info
                if si is not None and len(si.on_wait) > 1:
                    changed = True
                    waits = list(si.on_wait)
                    for w in waits[:-1]:
                        n += 1
                        out.append(mybir.InstNoOp(
                            name=f"splitwait-{n}", engine=ins.engine,
                            ins=[], outs=[], bass_nofuse=True,
                            sync_info=mybir.SyncInfo(on_wait=[w], on_update=[]),
                        ))
                    si.on_wait = waits[-1:]
                out.append(ins)
            if changed:
                bb.instructions = out
    return n


class Cfg:
    def __init__(self, n_nodes=100000, n_edges=1600000, in_dim=128,
                 hid_dim=64, out_dim=64, n_cores=8, proj_tile=512,
                 batch_cols=64, batch_groups=16, dspan=64, kcap=24, gq=4):
        self.n_nodes = n_nodes
        self.n_edges = n_edges
        self.in_dim = in_dim
        self.hid_dim = hid_dim
        self.out_dim = out_dim
        self.n_cores = n_cores
        self.proj_tile = proj_tile
        self.batch_cols = batch_cols    # max gather columns per batch
        self.batch_groups = batch_groups  # max groups per batch
        self.dspan = dspan              # groups per d-row gather span
        self.kcap = kcap                # max edges per slot
        self.gq = gq                    # SWDGE queues to round-robin
        self.dbg = set()                # debug feature kill-switches
        self.slab = ((n_nodes + n_cores - 1) // n_cores + 255) // 256 * 256
        self.ch = 2 * self.slab         # table chunk (int16-addressable)
        assert self.ch <= 32767
        self.n_chunks = (n_nodes + self.ch - 1) // self.ch
        self.s_stride = self.ch + 1     # chunk rows incl. sentinel row
        self.trows = self.n_chunks * self.s_stride
        assert self.ch % proj_tile == 0


def _host_plan(cfg: Cfg, src: np.ndarray, dst: np.ndarray):
    N, C, CH = cfg.n_nodes, cfg.n_cores, cfg.ch
    src = src.astype(np.int64)
    dst = dst.astype(np.int64)
    NQ = cfg.n_chunks

    schunk = src // CH
    order = np.lexsort((schunk, dst))     # edges by (dst, src-chunk)
    src_l = (src - schunk * CH)[order]    # chunk-local src per edge
    key = dst * NQ + schunk
    cnt = np.bincount(key[order], minlength=N * NQ)
    kstart = np.zeros(N * NQ + 1, np.int64)
    np.cumsum(cnt, out=kstart[1:])

    # slots: (core, chunk, node, len, estart); chop to <= kcap edges
    nz = np.nonzero(cnt)[0]
    kcap = min(cfg.kcap, cfg.batch_cols)
    nsub = (cnt[nz] + kcap - 1) // kcap
    cum = np.concatenate([[0], np.cumsum(nsub)])
    rep = np.repeat(np.arange(len(nz)), nsub)
    sub_off = (np.arange(len(rep)) - cum[rep]) * kcap
    s_node = nz[rep] // NQ
    s_chunk = nz[rep] % NQ
    s_len = np.minimum(cnt[nz][rep] - sub_off, kcap)
    s_start = kstart[nz][rep] + sub_off
    s_core = s_node // cfg.slab
    assert s_len.max() <= cfg.batch_cols, s_len.max()

    # per (core, chunk): sort slots by len asc
    percc = {}
    for c in range(C):
        for q in range(NQ):
            m = (s_core == c) & (s_chunk == q)
            o = np.argsort(s_len[m], kind="stable")
            percc[c, q] = (s_node[m][o], s_len[m][o], s_start[m][o])

    # global group structure: per chunk, ngq = max over cores
    ngq = [max((len(percc[c, q][0]) + P - 1) // P for c in range(C))
           for q in range(NQ)]
    ng = sum(ngq)
    group_chunk = np.concatenate(
        [np.full(ngq[q], q, np.int64) for q in range(NQ)])
    gq_base = np.concatenate([[0], np.cumsum(ngq)])

    # k_g = max slot len in group g across cores (>=1)
    k_g = np.ones(ng, np.int64)
    for q in range(NQ):
        for c in range(C):
            ln = percc[c, q][1]
            nslq = ngq[q] * P
            pad = np.zeros(nslq, np.int64)
            pad[:len(ln)] = ln
            k_g[gq_base[q]:gq_base[q + 1]] = np.maximum(
                k_g[gq_base[q]:gq_base[q + 1]], pad.reshape(ngq[q], P).max(1))
    offs = np.zeros(ng + 1, np.int64)
    np.cumsum(k_g, out=offs[1:])
    ktot = int(offs[-1])

    # batches: consecutive same-chunk groups, <= batch_cols columns
    batches = []  # (chunk, g_lo, g_hi, col_off)
    g = 0
    col = 0
    while g < ng:
        q = group_chunk[g]
        g2 = g
        cols = 0
        while (g2 < ng and group_chunk[g2] == q and g2 - g < cfg.batch_groups
               and cols + k_g[g2] <= cfg.batch_cols):
            cols += k_g[g2]
            g2 += 1
        assert g2 > g, f"group {g} width {k_g[g]} exceeds batch_cols"
        batches.append((int(q), g, g2, col))
        col += cols
        g = g2
    assert col == ktot

    # d spans: runs of <= dspan groups
    dspans = []
    g = 0
    while g < ng:
        g2 = min(g + cfg.dspan, ng)
        dspans.append((g, g2))
        g = g2

    def wrap16(flat):
        # dma_gather idx layout: index i at [i%16, i//16], tiled over 128
        b = flat.reshape(-1, 16).T
        return np.tile(b, (8, 1))

    # per-core arrays
    eidx, didx, dcnt, slot_nodes = [], [], [], []
    sent = CH  # chunk-local sentinel row
    for c in range(C):
        snode = np.full(ng * P, -1, np.int64)
        e_flat = np.full((ktot, P), sent, np.int64)  # [col, p]
        for q in range(NQ):
            nid, ln, st = percc[c, q]
            ns = len(nid)
            if ns == 0:
                continue
            sl = np.arange(ns)
            gg = gq_base[q] + sl // P
            pp = sl % P
            snode[gg * P + pp] = nid
            rep = np.repeat(sl, ln)
            jj = np.arange(rep.size) - np.repeat(
                np.concatenate([[0], np.cumsum(ln)])[:-1], ln)
            e_pos = np.repeat(st, ln) + jj
            e_flat[offs[gg[rep]] + jj, pp[rep]] = src_l[e_pos]
        # per batch: flat i = c*128 + p ordering, then 16-wrap
        eb = [wrap16(e_flat[b[3]:b[3] + int(offs[b[2]] - offs[b[1]])].ravel())
              for b in batches]
        eidx.append(np.concatenate(eb, axis=1).astype(np.int16))

        # d idx: per chunk pass q, per span: slot (g, p) -> local node
        qc = c // 2  # this core's slab chunk
        db, cb = [], []
        for q in range(NQ):
            for (glo, ghi) in dspans:
                nsl = (ghi - glo) * P
                if q == qc:
                    fl = snode[glo * P:ghi * P].copy()
                    fl = np.where(fl >= 0, fl - qc * CH, sent)
                    cb.append(nsl)
                elif "static_d" in cfg.dbg:
                    fl = np.full(nsl, sent, np.int64)
                    cb.append(nsl)
                else:
                    fl = np.full(nsl, -1, np.int64)
                    cb.append(0)
                db.append(wrap16(fl))
        didx.append(np.concatenate(db, axis=1).astype(np.int16))
        dcnt.append(np.array(cb, np.int32).reshape(1, -1))
        slot_nodes.append(snode)

    return {
        "ng": ng, "ktot": ktot, "k_g": k_g, "offs": offs,
        "batches": batches, "dspans": dspans,
        "eidx": eidx, "didx": didx, "dcnt": dcnt, "slot_nodes": slot_nodes,
    }


def _build_program(cfg: Cfg, plan) -> bass.Bass:
    _patch_tile_drain()
    N, D, H, IND = cfg.n_nodes, cfg.out_dim, cfg.hid_dim, cfg.in_dim
    NQ, CH, S = cfg.n_chunks, cfg.ch, cfg.s_stride
    ng, ktot = plan["ng"], plan["ktot"]
    k_g, offs = plan["k_g"], plan["offs"]
    batches, dspans = plan["batches"], plan["dspans"]
    TROW = 128
    PT = cfg.proj_tile
    nspans = len(dspans)

    nc = bacc.Bacc("TRN2", target_bir_lowering=False,
                   num_swdge_queues=cfg.gq)
    x_d = nc.dram_tensor("x", [N, IND], F16, kind="ExternalInput")
    win_d = nc.dram_tensor("w_in", [IND, H], F16, kind="ExternalInput")
    b_d = nc.dram_tensor("b_in", [H, 1], F32, kind="ExternalInput")
    w_d = nc.dram_tensor("w", [H, D], F16, kind="ExternalInput")
    wt_d = nc.dram_tensor("wT", [D, H], F16, kind="ExternalInput")
    a2_d = nc.dram_tensor("a2", [D, 2], F16, kind="ExternalInput")
    eidx_d = nc.dram_tensor("eidx", [P, 8 * ktot], I16, kind="ExternalInput")
    didx_d = nc.dram_tensor("didx", [P, plan["didx"][0].shape[1]], I16,
                            kind="ExternalInput")
    dcnt_d = nc.dram_tensor("dcnt", [1, NQ * nspans], I32, kind="ExternalInput")
    table_d = nc.dram_tensor("table", [cfg.trows, TROW], F16)
    out_d = nc.dram_tensor("out", [ng * P, D + 1], F32, kind="ExternalOutput")

    with tile.TileContext(nc) as tc:
        with (
            tc.tile_pool(name="const", bufs=1) as cpool,
            tc.tile_pool(name="psum", bufs=2, space="PSUM") as psum,
        ):
            # ---- constants ----
            win_sb = cpool.tile([IND, H], F16)
            nc.sync.dma_start(out=win_sb[:], in_=win_d[:])
            b_sb = cpool.tile([H, 1], F32)
            nc.sync.dma_start(out=b_sb[:], in_=b_d[:])
            rhs_sb = cpool.tile([H, D + 2], F16)  # [w | w@a0 | w@a1]
            nc.sync.dma_start(out=rhs_sb[:, 0:D], in_=w_d[:])
            wt_sb = cpool.tile([D, H], F16)
            nc.sync.dma_start(out=wt_sb[:], in_=wt_d[:])
            a2_sb = cpool.tile([D, 2], F16)
            nc.sync.dma_start(out=a2_sb[:], in_=a2_d[:])
            wa_ps = psum.tile([H, 2], F32, space="PSUM", tag="wa")
            nc.tensor.matmul(out=wa_ps[:], lhsT=wt_sb[:], rhs=a2_sb[:],
                             start=True, stop=True)
            nc.scalar.copy(out=rhs_sb[:, D:D + 2], in_=wa_ps[:])

            # sentinel rows (one per chunk): z = 0, s = SENT_S, d = 0
            sent_sb = cpool.tile([1, TROW], F16)
            nc.vector.memset(sent_sb[:], 0.0)
            nc.vector.memset(sent_sb[0:1, D:D + 1], SENT_S)
            for q in range(NQ):
                nc.sync.dma_start(out=table_d[q * S + CH:q * S + CH + 1, :],
                                  in_=sent_sb[:])

            # ---- phase A: projection + table ----
            with tc.tile_pool(name="proj", bufs=3) as proj:
                for t0 in range(0, N, PT):
                    tn = min(PT, N - t0)
                    q = t0 // CH
                    xt = proj.tile([IND, PT], F16, tag="xt")
                    nc.sync.dma_start_transpose(out=xt[:, :tn],
                                                in_=x_d[t0:t0 + tn, :])
                    h0_ps = psum.tile([H, PT], F32, space="PSUM", tag="h0")
                    for q0 in range(0, tn, 512):
                        qn = min(512, tn - q0)
                        nc.tensor.matmul(out=h0_ps[:, q0:q0 + qn],
                                         lhsT=win_sb[:], rhs=xt[:, q0:q0 + qn],
                                         start=True, stop=True)
                    h0_sb = proj.tile([H, PT], F16, tag="h0sb")
                    nc.scalar.activation(out=h0_sb[:, :tn], in_=h0_ps[:, :tn],
                                         func=AF.Gelu, bias=b_sb[:], scale=1.0)
                    nsub = (tn + P - 1) // P
                    zsd_ps = psum.tile([P, (PT // P) * (D + 2)], F32,
                                       space="PSUM", tag="zsd")
                    for c in range(nsub):
                        q0 = c * P
                        qn = min(P, tn - q0)
                        nc.tensor.matmul(
                            out=zsd_ps[:qn, c * (D + 2):(c + 1) * (D + 2)],
                            lhsT=h0_sb[:, q0:q0 + qn],
                            rhs=rhs_sb[:], start=True, stop=True)
                    stage = proj.tile([P, PT // P, TROW], F16, tag="stage")
                    if tn == PT:
                        nc.scalar.copy(
                            out=stage[:, :, 0:D + 2],
                            in_=zsd_ps[:].rearrange("p (c e) -> p c e",
                                                    e=D + 2))
                        nc.sync.dma_start(
                            out=table_d[t0 + q:t0 + q + tn, :].rearrange(
                                "(c p) f -> p c f", p=P),
                            in_=stage[:])
                    else:
                        for c in range(nsub):
                            q0 = c * P
                            qn = min(P, tn - q0)
                            nc.scalar.copy(
                                out=stage[:qn, c, 0:D + 2],
                                in_=zsd_ps[:qn, c * (D + 2):(c + 1) * (D + 2)])
                            nc.sync.dma_start(
                                out=table_d[t0 + q0 + q:t0 + q0 + q + qn, :],
                                in_=stage[:qn, c, :])

            # Phase B gathers are ordered after the table writes they read
            # via DRAM RAW dependency tracking (no all-engine barrier, so
            # early-chunk gathers overlap late-chunk projection).

            # ---- phase B ----
            eidx_sb = cpool.tile([P, 8 * ktot], I16)
            nc.sync.dma_start(out=eidx_sb[:], in_=eidx_d[:])
            didx_sb = cpool.tile([P, plan["didx"][0].shape[1]], I16)
            nc.sync.dma_start(out=didx_sb[:], in_=didx_d[:])
            dcnt_sb = cpool.tile([1, NQ * nspans], I32)
            nc.sync.dma_start(out=dcnt_sb[:], in_=dcnt_d[:])
            d_all = cpool.tile([P, ng], F32)

            with (
                tc.tile_pool(name="dpool", bufs=2) as dpool,
                tc.tile_pool(name="epool",
                             bufs=1 if "serial_bt" in cfg.dbg else 3) as epool,
                tc.tile_pool(name="spool", bufs=3) as spool,
                tc.tile_pool(name="rpool", bufs=cfg.batch_groups + 2) as rpool,
            ):
                # d rows: one pass per chunk into a shared span buffer;
                # inactive passes (per-core data: num_idxs_reg=0 + all -1
                # idxs) generate no descriptors, so exactly the active
                # chunk fills each slot's row.
                dmax = max(g2 - g1 for (g1, g2) in dspans)
                qctr = [0]  # rotate every gather across the SWDGE queues

                def next_q():
                    qn = qctr[0] % cfg.gq
                    qctr[0] += 1
                    return qn

                if "no_dgather" not in cfg.dbg:
                    dreg = (None if "static_d" in cfg.dbg
                            else nc.gpsimd.alloc_register())
                    for sp_i, (glo, ghi) in enumerate(dspans):
                        gn = ghi - glo
                        drows = dpool.tile([P, dmax, TROW], F16, tag="dr")
                        for q in range(NQ):
                            si = q * len(dspans) + sp_i
                            co = 8 * (q * ng + glo)
                            if dreg is None:
                                reg = gn * P
                            else:
                                reg = dreg
                                nc.gpsimd.reg_load(dreg,
                                                   dcnt_sb[0:1, si:si + 1])
                            nc.gpsimd.dma_gather(
                                out_ap=drows[:, :gn, :],
                                in_ap=table_d[q * S:q * S + S, :],
                                idxs_ap=didx_sb[:, co:co + 8 * gn],
                                num_idxs=gn * P, num_idxs_reg=reg,
                                elem_size=TROW,
                                single_packet=gn * P <= 1008,
                                queue_num=next_q())
                        nc.vector.tensor_copy(out=d_all[:, glo:ghi],
                                              in_=drows[:, :gn, D + 1])
                else:
                    nc.vector.memset(d_all[:], 0.0)

                # edge batches
                for b_i, (q, g1, g2, coff) in enumerate(
                        [] if "no_batches" in cfg.dbg else batches):
                    cols = int(offs[g2] - offs[g1])
                    bt = epool.tile([P, cfg.batch_cols, TROW], F16, tag="bt")
                    nc.gpsimd.dma_gather(
                        out_ap=bt[:, :cols, :],
                        in_ap=table_d[q * S:q * S + S, :],
                        idxs_ap=eidx_sb[:, 8 * coff:8 * (coff + cols)],
                        num_idxs=cols * P, num_idxs_reg=cols * P,
                        elem_size=TROW, single_packet=cols * P <= 1008,
                        queue_num=next_q())
                    if "no_compute" in cfg.dbg:
                        continue
                    wexp = spool.tile([P, cfg.batch_cols, 1], F16, tag="wx")
                    res_list = []
                    for g in range(g1, g2):
                        k = int(k_g[g])
                        lo = int(offs[g] - offs[g1])
                        # lrelu(s + d) on DVE: Lrelu's ACT table set differs
                        # from Exp's, and alternating them reloads the LUT
                        # (~1us) twice per group.
                        tt = spool.tile([P, k, 1], F32, tag="tt")
                        nc.vector.tensor_scalar(
                            out=tt[:], in0=bt[:, lo:lo + k, D:D + 1],
                            scalar1=d_all[:, g:g + 1], scalar2=None,
                            op0=ALU.add)
                        ew = spool.tile([P, k, 1], F32, tag="ew")
                        nc.vector.scalar_tensor_tensor(
                            out=ew[:], in0=tt[:], scalar=0.01, in1=tt[:],
                            op0=ALU.mult, op1=ALU.max)
                        res = rpool.tile([P, D + 1], F32, tag="res")
                        res_list.append(res)
                        nc.scalar.activation(
                            out=wexp[:, lo:lo + k, :], in_=ew[:],
                            func=AF.Exp, accum_out=res[:, D:D + 1])
                    msg = spool.tile([P, cfg.batch_cols, D], F16, tag="msg")
                    nc.vector.tensor_tensor(
                        out=msg[:, :cols, :], in0=bt[:, :cols, 0:D],
                        in1=wexp[:, :cols, :].to_broadcast([P, cols, D]),
                        op=ALU.mult)
                    for g in range(g1, g2):
                        k = int(k_g[g])
                        lo = int(offs[g] - offs[g1])
                        res = res_list[g - g1]
                        nc.vector.tensor_reduce(
                            out=res[:, 0:D],
                            in_=msg[:, lo:lo + k, :].rearrange("p k f -> p f k"),
                            axis=mybir.AxisListType.X, op=ALU.add)
                        nc.sync.dma_start(out=out_d[g * P:(g + 1) * P, :],
                                          in_=res[:])
    return nc


def _make_in_maps(cfg: Cfg, plan, x, w_in, b_in, w, a):
    x16 = np.asarray(x, np.float16)
    win16 = np.asarray(w_in, np.float16)
    b32 = np.asarray(b_in, np.float32).reshape(cfg.hid_dim, 1)
    w16 = np.asarray(w, np.float16)
    wt16 = np.ascontiguousarray(np.asarray(w).T).astype(np.float16)
    a = np.asarray(a)
    a2 = np.stack([a[:cfg.out_dim], a[cfg.out_dim:]], axis=1).astype(np.float16)
    in_maps = []
    for c in range(cfg.n_cores):
        in_maps.append({
            "x": x16, "w_in": win16, "b_in": b32, "w": w16, "wT": wt16,
            "a2": a2, "eidx": plan["eidx"][c], "didx": plan["didx"][c],
            "dcnt": plan["dcnt"][c],
        })
    return in_maps


def _run_cores(cfg: Cfg, plan, x, w_in, b_in, w, a, trace=False):
    nc = _build_program(cfg, plan)
    nc.finalize()
    _split_sync_waits(nc)
    in_maps = _make_in_maps(cfg, plan, x, w_in, b_in, w, a)
    return run_bass_kernel_spmd(nc, in_maps, list(range(cfg.n_cores)),
                                trace=trace)


def kernel(x, w_in, b_in, w, a, src, dst, cfg: Cfg = None, _res_hook=None,
           _trace=False):
    cfg = cfg or Cfg()
    src = np.asarray(src)
    dst = np.asarray(dst)

    plan = _host_plan(cfg, src, dst)
    res = _run_cores(cfg, plan, x, w_in, b_in, w, a, trace=_trace)
    if _res_hook is not None:
        _res_hook(res)

    D = cfg.out_dim
    U = np.zeros((cfg.n_nodes, D), np.float64)
    den = np.zeros(cfg.n_nodes, np.float64)
    for c in range(cfg.n_cores):
        out = np.asarray(res.results[c]["out"], np.float64)
        snode = plan["slot_nodes"][c]
        m = snode >= 0
        np.add.at(U, snode[m], out[m, :D])
        np.add.at(den, snode[m], out[m, D])
    h = U / np.maximum(den, 1e-9)[:, None]
    return h.astype(np.float32)

